# revision 8
# baseline (speedup 1.0000x reference)
"""Trainium2 Bass kernel for the batched natural-cubic-spline + MLP model.

Math: the whole spline pipeline (coeff construction via a constant tridiagonal
solve, evaluation at t = sigmoid(raw_index)) is linear in x, so
    outputs = x @ E,   E (N x T) built from raw_index only:
       E[:, j] = c0_j*onehot(i_j) + c1_j*onehot(i_j+1) + c2_j*K[:, i_j] + c3_j*K[:, i_j+1]
where kd = x @ K and K = R @ inv(Tri) is an input-independent constant
(precomputed on host in float64 — depends only on n_bands).  Folding with the
first MLP layer, M1 = E @ W1 (N x 50), the dominant device work is
    h1 = leaky(x @ M1 + b1)  ->  tiny MLP tail.

Distribution: data-parallel over batch (1024 rows/core).  Each core builds
E^T restricted to a 250-column slice (via one indirect-DMA row gather from a
per-core [K | I] pair table), computes its 250-row slice of M1, and an
AllGather assembles the full M1 on every core.  x is supplied band-major
(x^T shards) so the main matmul contracts over bands on partitions.
"""

import functools

import numpy as np

N = 2000          # bands (spline knots)
T = 500           # eval points
BATCH = 8192
NCORES = 8
BPC = BATCH // NCORES      # 1024 batch rows per core
SL = N // NCORES           # 250: M1 row-slice per core
HID = 50
HID2 = 10
H = 1.0 / (N - 1)
JC = 4            # j-chunks (T = 4*125)
JP = T // JC      # 125 partitions per j-chunk
KT = 16           # band chunks (15*128 + 80)


# ----------------------------------------------------------------- host math
@functools.lru_cache(maxsize=1)
def _tables():
    """K (f64) and the per-core gather tables KI[p] (N x 4*SL) f32."""
    hr = float(N - 1)
    main = np.full(N, 4.0 * hr)
    main[0] = main[-1] = 2.0 * hr
    off = np.full(N - 1, hr)
    A = np.diag(main) + np.diag(off, 1) + np.diag(off, -1)
    A_inv = np.linalg.inv(A)
    R = np.zeros((N, N))
    c = 3.0 * hr * hr
    idx = np.arange(N)
    # rhs_i gets +c at x_{i+1}, -c at x_i (for i<=N-2); +c at x_i, -c at x_{i-1} (i>=1)
    R[idx[:-1] + 1, idx[:-1]] += c
    R[idx[:-1], idx[:-1]] -= c
    R[idx[1:], idx[1:]] += c
    R[idx[1:] - 1, idx[1:]] -= c
    K = (R @ A_inv).astype(np.float32)

    tables = []
    for p in range(NCORES):
        sl = slice(SL * p, SL * p + SL)
        KI = np.zeros((N, 4 * SL), np.float32)
        # rows 0..N-2 are valid gather targets (idx <= N-2)
        KI[: N - 1, 0:SL] = K[sl, : N - 1].T            # K[:, i] slice
        KI[: N - 1, SL:2 * SL] = K[sl, 1:N].T           # K[:, i+1] slice
        ii = np.arange(N - 1)
        loc = ii - SL * p
        m = (loc >= 0) & (loc < SL)
        KI[ii[m], 2 * SL + loc[m]] = 1.0                # onehot(i)
        loc1 = ii + 1 - SL * p
        m1 = (loc1 >= 0) & (loc1 < SL)
        KI[ii[m1], 3 * SL + loc1[m1]] = 1.0             # onehot(i+1)
        tables.append(KI)
    return tables


# ----------------------------------------------------------------- bass graph
@functools.lru_cache(maxsize=1)
def _build_nc():
    from contextlib import ExitStack

    from concourse import bacc, bass, mybir, tile

    f32 = mybir.dt.float32
    f32r = mybir.dt.float32r
    i32 = mybir.dt.int32
    Id = mybir.ActivationFunctionType.Identity
    Sig = mybir.ActivationFunctionType.Sigmoid
    op = mybir.AluOpType

    nc = bacc.Bacc(None, num_devices=NCORES, num_swdge_queues=4)

    xt = nc.declare_dram_parameter("xt", [N, BPC], f32, isOutput=False)
    kip = nc.declare_dram_parameter("kip", [N, 4 * SL], f32, isOutput=False)
    raw = nc.declare_dram_parameter("raw_index", [T], f32, isOutput=False)
    w1 = nc.declare_dram_parameter("w1", [T, HID], f32, isOutput=False)
    b1 = nc.declare_dram_parameter("b1", [HID], f32, isOutput=False)
    w2 = nc.declare_dram_parameter("w2", [HID, HID2], f32, isOutput=False)
    b2 = nc.declare_dram_parameter("b2", [HID2], f32, isOutput=False)
    w3 = nc.declare_dram_parameter("w3", [HID2, 1], f32, isOutput=False)
    b3 = nc.declare_dram_parameter("b3", [1], f32, isOutput=False)
    out = nc.declare_dram_parameter("out", [BPC], f32, isOutput=True)

    ctx = ExitStack()
    with ctx:
        tc = ctx.enter_context(tile.TileContext(nc))
        sb = ctx.enter_context(tc.tile_pool(name="sb", bufs=1))
        ps = ctx.enter_context(tc.tile_pool(name="ps", bufs=1, space="PSUM"))
        dr = ctx.enter_context(tc.tile_pool(name="dr", bufs=1, space="DRAM"))

        def stile(shape, dtype, tag):
            return sb.tile(shape, dtype, tag=tag, name=tag)

        # ---- small parameter loads (SWDGE so they never queue behind x)
        raw_sb = stile([JP, JC], f32, "raw")
        nc.gpsimd.dma_start(out=raw_sb[:], in_=raw[:].rearrange("(c p) -> p c", p=JP))
        w1_sb = stile([JP, JC, HID], f32, "w1")
        nc.gpsimd.dma_start(
            out=w1_sb[:], in_=w1[:, :].rearrange("(c p) o -> p c o", p=JP)
        )
        w2_sb = stile([HID, HID2], f32r, "w2")
        nc.gpsimd.dma_start(out=w2_sb[:], in_=w2[:, :])
        w3_sb = stile([HID2, 1], f32r, "w3")
        nc.gpsimd.dma_start(out=w3_sb[:], in_=w3[:, :])
        b1_sb = stile([HID, 1], f32, "b1")
        nc.gpsimd.dma_start(out=b1_sb[:], in_=b1[:].rearrange("(p a) -> p a", a=1))
        b2_sb = stile([HID2, 1], f32, "b2")
        nc.gpsimd.dma_start(out=b2_sb[:], in_=b2[:].rearrange("(p a) -> p a", a=1))
        b3_sb = stile([1, 1], f32, "b3")
        nc.gpsimd.dma_start(out=b3_sb[:], in_=b3[:].rearrange("(p a) -> p a", a=1))

        # ---- spline interval + cubic coefficients (all [JP, JC])
        def vtile(tag):
            return stile([JP, JC], f32, tag)

        t_sb = vtile("t")
        nc.scalar.activation(t_sb[:], raw_sb[:], Sig)
        tn = vtile("tn")
        nc.vector.tensor_scalar_mul(tn[:], t_sb[:], float(N - 1))
        ii_t = stile([JP, JC], i32, "iit")
        nc.vector.tensor_copy(out=ii_t[:], in_=tn[:])
        iff = vtile("iff")
        nc.vector.tensor_copy(out=iff[:], in_=ii_t[:])
        gtm = vtile("gtm")
        nc.vector.tensor_tensor(out=gtm[:], in0=iff[:], in1=tn[:], op=op.is_gt)
        idxf = vtile("idxf")
        nc.vector.tensor_tensor(out=idxf[:], in0=iff[:], in1=gtm[:], op=op.subtract)
        idxc = vtile("idxc")
        nc.vector.tensor_scalar(idxc[:], idxf[:], float(N - 2), 0.0, op.min, op.max)
        u = vtile("u")
        nc.vector.tensor_tensor(out=u[:], in0=tn[:], in1=idxc[:], op=op.subtract)
        u2 = vtile("u2")
        nc.vector.tensor_tensor(out=u2[:], in0=u[:], in1=u[:], op=op.mult)
        um1 = vtile("um1")
        nc.vector.tensor_scalar(um1[:], u[:], 1.0, None, op.subtract)
        um1sq = vtile("um1sq")
        nc.vector.tensor_tensor(out=um1sq[:], in0=um1[:], in1=um1[:], op=op.mult)
        w32u = vtile("w32u")  # 3 - 2u
        nc.vector.tensor_scalar(w32u[:], u[:], -2.0, 3.0, op.mult, op.add)
        c1 = vtile("c1")
        nc.vector.tensor_tensor(out=c1[:], in0=u2[:], in1=w32u[:], op=op.mult)
        c0 = vtile("c0")
        nc.vector.tensor_scalar(c0[:], c1[:], -1.0, 1.0, op.mult, op.add)
        c2a = vtile("c2a")
        nc.vector.tensor_tensor(out=c2a[:], in0=u[:], in1=um1sq[:], op=op.mult)
        c2 = vtile("c2")
        nc.vector.tensor_scalar_mul(c2[:], c2a[:], H)
        c3a = vtile("c3a")
        nc.vector.tensor_tensor(out=c3a[:], in0=u2[:], in1=um1[:], op=op.mult)
        c3 = vtile("c3")
        nc.vector.tensor_scalar_mul(c3[:], c3a[:], H)
        idx_i = stile([JP, JC], i32, "idxi")
        nc.vector.tensor_copy(out=idx_i[:], in_=idxc[:])

        # ---- gather + combine -> E^T slice tiles (JP x 2*SL per chunk? no: JP x SL)
        et = []
        for c in range(JC):
            g = stile([JP, 4 * SL], f32, f"g{c}")
            nc.gpsimd.indirect_dma_start(
                out=g[:],
                out_offset=None,
                in_=kip[:, :],
                in_offset=bass.IndirectOffsetOnAxis(ap=idx_i[:, c : c + 1], axis=0),
            )
            ta = stile([JP, SL], f32, "cmb_a")
            tb = stile([JP, SL], f32, "cmb_b")
            e = stile([JP, SL], f32, f"et{c}")
            nc.vector.tensor_scalar_mul(ta[:], g[:, 0:SL], c2[:, c : c + 1])
            nc.vector.tensor_scalar_mul(tb[:], g[:, SL : 2 * SL], c3[:, c : c + 1])
            nc.vector.tensor_tensor(out=e[:], in0=ta[:], in1=tb[:], op=op.add)
            nc.vector.tensor_scalar_mul(
                ta[:], g[:, 2 * SL : 3 * SL], c0[:, c : c + 1]
            )
            nc.vector.tensor_tensor(out=e[:], in0=e[:], in1=ta[:], op=op.add)
            nc.vector.tensor_scalar_mul(
                tb[:], g[:, 3 * SL : 4 * SL], c1[:, c : c + 1]
            )
            nc.vector.tensor_tensor(out=e[:], in0=e[:], in1=tb[:], op=op.add)
            et.append(e)

        # ---- M1 slice (SL x HID) = E^T-slice.T @ W1, in two 125-row halves
        m1sl_sb = stile([JP, 2 * HID], f32, "m1sl")
        for hh in range(2):
            pt = ps.tile([JP, HID], f32, tag=f"m1ps{hh}", name=f"m1ps{hh}")
            for c in range(JC):
                nc.tensor.matmul(
                    pt[:],
                    lhsT=et[c][:, JP * hh : JP * hh + JP],
                    rhs=w1_sb[:, c, :],
                    start=(c == 0),
                    stop=(c == JC - 1),
                )
            nc.scalar.copy(out=m1sl_sb[:, HID * hh : HID * hh + HID], in_=pt[:])

        # ---- AllGather M1 (SL x HID per core -> N x HID everywhere)
        bounce_in = dr.tile([SL, HID], f32, tag="bin", name="bin")
        bounce_out = dr.tile([N, HID], f32, tag="bout", name="bout")
        nc.gpsimd.dma_start(
            out=bounce_in[:].rearrange("(h p) o -> p h o", h=2),
            in_=m1sl_sb[:].rearrange("p (h o) -> p h o", h=2),
        )
        nc.gpsimd.collective_compute(
            "AllGather",
            op.bypass,
            replica_groups=[list(range(NCORES))],
            ins=[bounce_in[:].opt()],
            outs=[bounce_out[:].opt()],
        )
        m1_sb = stile([128, KT, HID], f32r, "m1")
        nc.gpsimd.dma_start(
            out=m1_sb[:, 0 : KT - 1, :],
            in_=bounce_out[0 : 15 * 128, :].rearrange("(k p) o -> p k o", p=128),
        )
        nc.gpsimd.dma_start(
            out=m1_sb[0:80, KT - 1, :], in_=bounce_out[15 * 128 : N, :]
        )

        # ---- x^T tiles (HWDGE)
        xt_t = []
        for k in range(KT):
            rows = 128 if k < KT - 1 else N - 128 * (KT - 1)
            xti = stile([128, BPC], f32r, f"xt{k}")
            nc.gpsimd.dma_start(out=xti[:rows, :], in_=xt[128 * k : 128 * k + rows, :])
            xt_t.append(xti)

        # ---- main matmul: h1preT (HID x BPC) += M1_k.T @ xT_k
        h1ps = [ps.tile([HID, 512], f32, tag=f"h1ps{nh}", name=f"h1ps{nh}") for nh in range(2)]
        for k in range(KT):
            rows = 128 if k < KT - 1 else N - 128 * (KT - 1)
            for nh in range(2):
                nc.tensor.matmul(
                    h1ps[nh][:],
                    lhsT=m1_sb[:rows, k, :],
                    rhs=xt_t[k][:rows, 512 * nh : 512 * nh + 512],
                    start=(k == 0),
                    stop=(k == KT - 1),
                )

        # ---- epilogue: leaky(v) = max(v, 0.01 v)
        def leaky(pre_ps, bias_sb, width, parts, tagp):
            a = stile([parts, BPC], f32, f"{tagp}a")
            for nh in range(2):
                nc.scalar.activation(
                    a[:, 512 * nh : 512 * nh + 512], pre_ps[nh][:], Id,
                    bias=bias_sb[:, 0:1],
                )
            s = stile([parts, BPC], f32, f"{tagp}s")
            nc.vector.tensor_scalar_mul(s[:], a[:], 0.01)
            h = stile([parts, BPC], f32r, f"{tagp}h")
            nc.vector.tensor_tensor(out=h[:], in0=a[:], in1=s[:], op=op.max)
            return h

        h1 = leaky(h1ps, b1_sb, 512, HID, "h1")

        h2ps = [ps.tile([HID2, 512], f32, tag=f"h2ps{nh}", name=f"h2ps{nh}") for nh in range(2)]
        for nh in range(2):
            nc.tensor.matmul(
                h2ps[nh][:],
                lhsT=w2_sb[:],
                rhs=h1[:, 512 * nh : 512 * nh + 512],
                start=True,
                stop=True,
            )
        h2 = leaky(h2ps, b2_sb, 512, HID2, "h2")

        yps = [ps.tile([1, 512], f32, tag=f"yps{nh}", name=f"yps{nh}") for nh in range(2)]
        y_sb = stile([1, BPC], f32, "y")
        for nh in range(2):
            nc.tensor.matmul(
                yps[nh][:],
                lhsT=w3_sb[:],
                rhs=h2[:, 512 * nh : 512 * nh + 512],
                start=True,
                stop=True,
            )
            nc.scalar.activation(
                y_sb[:, 512 * nh : 512 * nh + 512], yps[nh][:], Id, bias=b3_sb[:, 0:1]
            )
        nc.gpsimd.dma_start(
            out=out[:].rearrange("(a b) -> a b", a=1), in_=y_sb[:]
        )

    return nc


# ------------------------------------------------------------------- driver
TRACE = False          # set by test harness to capture a profile
LAST_RESULT = None     # BassKernelResults of the last run (when TRACE)


def kernel(x, raw_index, W1, b1, W2, b2, W3, b3):
    global LAST_RESULT
    from concourse.bass_utils import run_bass_kernel_spmd

    x = np.ascontiguousarray(x, np.float32)
    tables = _tables()
    nc = _build_nc()
    if not nc.is_finalized():
        nc.finalize()
    in_maps = []
    for p in range(NCORES):
        in_maps.append(
            {
                "xt": np.ascontiguousarray(x[BPC * p : BPC * (p + 1)].T),
                "kip": tables[p],
                "raw_index": np.ascontiguousarray(raw_index, np.float32),
                "w1": np.ascontiguousarray(W1, np.float32),
                "b1": np.ascontiguousarray(b1, np.float32),
                "w2": np.ascontiguousarray(W2, np.float32),
                "b2": np.ascontiguousarray(b2, np.float32),
                "w3": np.ascontiguousarray(W3, np.float32),
                "b3": np.ascontiguousarray(b3, np.float32),
            }
        )
    res = run_bass_kernel_spmd(
        nc, in_maps, core_ids=list(range(NCORES)), trace=TRACE
    )
    if TRACE:
        LAST_RESULT = res
    return np.concatenate(
        [np.asarray(res.results[p]["out"]).ravel() for p in range(NCORES)]
    )


# revision 19
# speedup vs baseline: 1.1073x; 1.1073x over previous
"""Trainium2 Bass kernel for the batched natural-cubic-spline + MLP model.

Math: the spline pipeline (coeff construction via a constant tridiagonal
solve, evaluation at t = sigmoid(raw_index)) is linear in x:
    outputs = x @ E,  E (N x T) with column j =
       c0*onehot(i_j) + c1*onehot(i_j+1) + c2*K[:, i_j] + c3*K[:, i_j+1]
where kd = x @ K and K = R @ inv(Tridiag) is input-independent (host f64
precompute) and BANDED (half-width 31 at fp32 precision).  Folding with the
first MLP layer, M1 = E @ W1 (N x 50), the device work is
    h1 = leaky(x @ M1 + b1) -> tiny MLP tail.

Per core (pure data-parallel, NO collectives):
  1. gather 64-wide K-band strips for its 500 eval points from a compact
     band table (indirect DMA),
  2. combine with the cubic coefficients -> E column strips (bf16),
  3. indirect-scatter the strips into a zeroed DRAM buffer -> dense E^T,
  4. reload E^T, fold with W1 on TensorE -> M1^T, transpose -> M1,
  5. h1^T = M1^T @ x^T (f32r matmuls, x supplied band-major), MLP tail.
Biases ride the matmuls via ones-rows appended to the moving operands.
"""

import functools

import numpy as np

N = 2000          # bands (spline knots)
T = 500           # eval points
BATCH = 8192
NCORES = 8
BPC = BATCH // NCORES      # 1024 batch rows per core
HID = 50
HID2 = 10
H = 1.0 / (N - 1)
JC = 4            # j-chunks (T = 4*125)
JP = T // JC      # 125 partitions per j-chunk
KT = 16           # band chunks (15*128 + 80)
W = 31            # band half-width; 64-wide windows
EW = 2048         # padded E^T row width
SMALL_W = 272     # packed small-input width


# ----------------------------------------------------------------- host math
@functools.lru_cache(maxsize=1)
def _band_table():
    """KB (N x 256) f32: per-knot [K-band(i) | K-band(i+1) | I-win(i) | I-win(i+1)]."""
    hr = float(N - 1)
    main = np.full(N, 4.0 * hr)
    main[0] = main[-1] = 2.0 * hr
    off = np.full(N - 1, hr)
    A = np.diag(main) + np.diag(off, 1) + np.diag(off, -1)
    A_inv = np.linalg.inv(A)
    R = np.zeros((N, N))
    c = 3.0 * hr * hr
    idx = np.arange(N)
    R[idx[:-1] + 1, idx[:-1]] += c
    R[idx[:-1], idx[:-1]] -= c
    R[idx[1:], idx[1:]] += c
    R[idx[1:] - 1, idx[1:]] -= c
    K = R @ A_inv  # f64

    KB = np.zeros((N, 256), np.float32)
    drop = 0.0
    for i in range(N - 1):
        s = min(max(i - W, 0), N - 64)
        d = np.arange(64)
        KB[i, 0:64] = K[s + d, i]
        KB[i, 64:128] = K[s + d, i + 1]
        KB[i, 128 + (i - s)] = 1.0
        KB[i, 192 + (i + 1 - s)] = 1.0
        # dropped off-window band mass (sanity)
        m = np.ones(N, bool)
        m[s : s + 64] = False
        drop = max(drop, np.abs(K[m, i]).max(), np.abs(K[m, i + 1]).max())
    assert drop < 1e-7 * np.abs(K).max(), drop
    sv = np.minimum(np.maximum(np.arange(N) - W, 0), N - 64)
    return KB, sv


def _pack_small(raw_index, W1, b1, W2, b2, W3, b3):
    """One (128 x SMALL_W) f32 array holding all small inputs."""
    P = np.zeros((128, SMALL_W), np.float32)
    P[0:JP, 0:JC] = raw_index.reshape(JC, JP).T
    P[0:JP, 4:204] = W1.reshape(JC, JP, HID).transpose(1, 0, 2).reshape(JP, JC * HID)
    P[0:HID, 204:214] = W2
    P[HID, 204:214] = b2                       # W2ext row 50
    P[0, 214:264] = b1                         # b1 as a row
    P[0:HID2, 264] = W3[:, 0]
    P[HID2, 264] = b3[0]                       # W3ext row 10
    jj = (np.arange(JC)[None, :] * JP + np.arange(JP)[:, None]).astype(np.float32)
    P[0:JP, 268:272] = jj * float(EW)          # flat row base j*EW
    return P


# ----------------------------------------------------------------- bass graph
@functools.lru_cache(maxsize=1)
def _build_nc():
    from contextlib import ExitStack

    from concourse import bacc, bass, mybir, tile
    from concourse.masks import make_identity

    f32 = mybir.dt.float32
    f32r = mybir.dt.float32r
    bf16 = mybir.dt.bfloat16
    i32 = mybir.dt.int32
    Id = mybir.ActivationFunctionType.Identity
    Sig = mybir.ActivationFunctionType.Sigmoid
    op = mybir.AluOpType

    nc = bacc.Bacc(None, num_devices=NCORES, num_swdge_queues=4)

    xt = nc.declare_dram_parameter("xt", [N, BPC], f32r, isOutput=False)
    kb = nc.declare_dram_parameter("kb", [N, 256], f32, isOutput=False)
    small = nc.declare_dram_parameter("small", [128, SMALL_W], f32, isOutput=False)
    out = nc.declare_dram_parameter("out", [BPC], f32, isOutput=True)

    ctx = ExitStack()
    with ctx:
        tc = ctx.enter_context(tile.TileContext(nc))
        sb = ctx.enter_context(tc.tile_pool(name="sb", bufs=1))
        pst = ctx.enter_context(tc.tile_pool(name="pst", bufs=2, space="PSUM"))
        psh = ctx.enter_context(tc.tile_pool(name="psh", bufs=1, space="PSUM"))
        dr = ctx.enter_context(tc.tile_pool(name="dr", bufs=1, space="DRAM"))

        def stile(shape, dtype, tag):
            return sb.tile(shape, dtype, tag=tag, name=tag)

        # ---- E^T dense buffers in DRAM (one per j-chunk), zeroed on-device
        etz = [dr.tile([JP, EW], bf16, tag=f"etz{c}", name=f"etz{c}") for c in range(JC)]
        etz_flat = [
            e[:].rearrange("a b -> (a b)").rearrange("(x y) -> x y", y=1) for e in etz
        ]
        zt = stile([JP, EW], bf16, "zt")
        nc.vector.memset(zt[:], 0.0)
        for c in range(JC):
            nc.sync.dma_start(out=etz[c][:, :], in_=zt[:])

        # ---- packed small-parameter load (one HWDGE DMA)
        small_sb = stile([128, SMALL_W], f32, "small")
        nc.sync.dma_start(out=small_sb[:], in_=small[:, :])
        raw_sb = small_sb[0:JP, 0:JC]
        w1_sb = small_sb[0:JP, 4:204].rearrange("p (c o) -> p c o", c=JC)
        w2ext = small_sb[0 : HID + 1, 204:214]
        b1row = small_sb[0:1, 214:264]
        w3ext = small_sb[0 : HID2 + 1, 264:265]
        rowbase = small_sb[0:JP, 268:269]

        # ---- x^T tiles (both HWDGE rings) + ones row
        ones_f = stile([1, BPC], f32, "onesf")
        nc.vector.memset(ones_f[:], 1.0)
        ones_t = stile([1, BPC], f32r, "ones")
        nc.vector.tensor_copy(out=ones_t[:], in_=ones_f[:])
        xt_t = []
        for k in range(KT):
            rows = 128 if k < KT - 1 else N - 128 * (KT - 1)
            xti = stile([128, BPC], f32r, f"xt{k}")
            eng = nc.sync if k % 2 == 0 else nc.scalar
            eng.dma_start(out=xti[:rows, :], in_=xt[128 * k : 128 * k + rows, :])
            xt_t.append(xti)

        # identity for PE transposes
        ident = stile([64, 64], f32, "ident")
        make_identity(nc, ident[:])

        # ---- spline interval + cubic coefficients (all [JP, JC])
        def vtile(tag):
            return stile([JP, JC], f32, tag)

        t_sb = vtile("t")
        nc.scalar.activation(t_sb[:], raw_sb, Sig)
        tn = vtile("tn")
        nc.vector.tensor_scalar_mul(tn[:], t_sb[:], float(N - 1))
        ii_t = stile([JP, JC], i32, "iit")
        nc.vector.tensor_copy(out=ii_t[:], in_=tn[:])
        iff = vtile("iff")
        nc.vector.tensor_copy(out=iff[:], in_=ii_t[:])
        gtm = vtile("gtm")
        nc.vector.tensor_tensor(out=gtm[:], in0=iff[:], in1=tn[:], op=op.is_gt)
        idxf = vtile("idxf")
        nc.vector.tensor_tensor(out=idxf[:], in0=iff[:], in1=gtm[:], op=op.subtract)
        idxc = vtile("idxc")
        nc.vector.tensor_scalar(idxc[:], idxf[:], float(N - 2), 0.0, op.min, op.max)
        idx_i = stile([JP, JC], i32, "idxi")
        nc.vector.tensor_copy(out=idx_i[:], in_=idxc[:])
        # scatter offsets: flat = j*EW + clip(idx-31, 0, N-64)
        sc0 = vtile("sc0")
        nc.vector.tensor_scalar(sc0[:], idxc[:], float(W), 0.0, op.subtract, op.max)
        sc1 = vtile("sc1")
        nc.vector.tensor_scalar(sc1[:], sc0[:], float(N - 64), None, op.min)
        flat = vtile("flat")
        nc.vector.tensor_scalar_add(flat[:], sc1[:], rowbase)
        flat_i = stile([JP, JC], i32, "flati")
        nc.vector.tensor_copy(out=flat_i[:], in_=flat[:])
        # cubic coefficients
        u = vtile("u")
        nc.vector.tensor_tensor(out=u[:], in0=tn[:], in1=idxc[:], op=op.subtract)
        u2 = vtile("u2")
        nc.vector.tensor_tensor(out=u2[:], in0=u[:], in1=u[:], op=op.mult)
        um1 = vtile("um1")
        nc.vector.tensor_scalar(um1[:], u[:], 1.0, None, op.subtract)
        um1sq = vtile("um1sq")
        nc.vector.tensor_tensor(out=um1sq[:], in0=um1[:], in1=um1[:], op=op.mult)
        w32u = vtile("w32u")  # 3 - 2u
        nc.vector.tensor_scalar(w32u[:], u[:], -2.0, 3.0, op.mult, op.add)
        c1 = vtile("c1")
        nc.vector.tensor_tensor(out=c1[:], in0=u2[:], in1=w32u[:], op=op.mult)
        c0 = vtile("c0")
        nc.vector.tensor_scalar(c0[:], c1[:], -1.0, 1.0, op.mult, op.add)
        c2a = vtile("c2a")
        nc.vector.tensor_tensor(out=c2a[:], in0=u[:], in1=um1sq[:], op=op.mult)
        c2 = vtile("c2")
        nc.vector.tensor_scalar_mul(c2[:], c2a[:], H)
        c3a = vtile("c3a")
        nc.vector.tensor_tensor(out=c3a[:], in0=u2[:], in1=um1[:], op=op.mult)
        c3 = vtile("c3")
        nc.vector.tensor_scalar_mul(c3[:], c3a[:], H)

        # ---- W1 chunks cast to bf16
        w1b = []
        for c in range(JC):
            wb = stile([JP, HID], bf16, f"w1b{c}")
            nc.vector.tensor_copy(out=wb[:], in_=w1_sb[:, c, :])
            w1b.append(wb)

        # ---- per chunk: gather strips, combine, scatter into etz
        for c in range(JC):
            g = stile([JP, 256], f32, f"g{c}")
            nc.gpsimd.indirect_dma_start(
                out=g[:],
                out_offset=None,
                in_=kb[:, :],
                in_offset=bass.IndirectOffsetOnAxis(ap=idx_i[:, c : c + 1], axis=0),
            )
            ta = stile([JP, 64], f32, f"cmb_a{c % 2}")
            tb = stile([JP, 64], f32, f"cmb_b{c % 2}")
            tcx = stile([JP, 64], f32, f"cmb_c{c % 2}")
            e_f = stile([JP, 64], f32, f"cmb_e{c % 2}")
            strip = stile([JP, 64], bf16, f"strip{c % 2}")
            nc.scalar.activation(ta[:], g[:, 0:64], Id, scale=c2[:, c : c + 1])
            nc.vector.tensor_scalar_mul(tb[:], g[:, 64:128], c3[:, c : c + 1])
            nc.scalar.activation(tcx[:], g[:, 128:192], Id, scale=c0[:, c : c + 1])
            nc.vector.scalar_tensor_tensor(
                out=e_f[:],
                in0=g[:, 192:256],
                scalar=c1[:, c : c + 1],
                in1=ta[:],
                op0=op.mult,
                op1=op.add,
            )
            nc.vector.tensor_tensor(out=tb[:], in0=tb[:], in1=tcx[:], op=op.add)
            nc.vector.tensor_tensor(out=strip[:], in0=e_f[:], in1=tb[:], op=op.add)
            nc.gpsimd.indirect_dma_start(
                out=etz_flat[c],
                out_offset=bass.IndirectOffsetOnAxis(
                    ap=flat_i[:, c : c + 1], axis=0
                ),
                in_=strip[:],
                in_offset=None,
            )

        # ---- reload dense E^T and fold with W1 -> M1^T (50 x 2000)
        psm_cm = tc.tile_pool(name="psm", bufs=1, space="PSUM")
        psm = psm_cm.__enter__()
        m1t_ps = psm.tile([HID, 4, 512], f32, tag="m1t", name="m1t")
        et_sb = []
        for c in range(JC):
            e2 = stile([JP, EW], bf16, f"et{c}")
            nc.scalar.dma_start(out=e2[:], in_=etz[c][:, :])
            et_sb.append(e2)
        for c in range(JC):
            for s in range(4):
                nc.tensor.matmul(
                    m1t_ps[:, s, 0:500],
                    lhsT=w1b[c][:],
                    rhs=et_sb[c][:, 500 * s : 500 * s + 500],
                    start=(c == 0),
                    stop=(c == JC - 1),
                )
        m1t_sb = stile([HID, N], f32, "m1ts")
        for s in range(4):
            if s % 2 == 0:
                nc.scalar.copy(
                    out=m1t_sb[:, 500 * s : 500 * s + 500], in_=m1t_ps[:, s, 0:500]
                )
            else:
                nc.vector.tensor_copy(
                    out=m1t_sb[:, 500 * s : 500 * s + 500], in_=m1t_ps[:, s, 0:500]
                )

        psm_cm.__exit__(None, None, None)

        # ---- transpose M1^T -> M1 (128 x (KT+1)*50), f32r; chunk KT is b1 row
        m1_sb = stile([128, (KT + 1) * HID], f32r, "m1")
        m1_v = m1_sb[:].rearrange("p (k o) -> p k o", o=HID)
        for k in range(KT):
            rows = 128 if k < KT - 1 else N - 128 * (KT - 1)
            ptr = pst.tile([128, HID], f32, tag="ptr", name=f"ptr{k}")
            nc.tensor.transpose(
                out=ptr[:rows, :],
                in_=m1t_sb[:, 128 * k : 128 * k + rows],
                identity=ident[0:HID, 0:HID],
            )
            if k % 2 == 0:
                nc.scalar.copy(out=m1_v[0:rows, k, :], in_=ptr[:rows, :])
            else:
                nc.vector.tensor_copy(out=m1_v[0:rows, k, :], in_=ptr[:rows, :])
        nc.scalar.copy(out=m1_v[0:1, KT, :], in_=b1row)

        # ---- main matmul: h1preT (HID x BPC) += M1_k.T @ xT_k (+ b1 via ones)
        h1ps = [
            psh.tile([HID, 512], f32, tag=f"h1ps{nh}", name=f"h1ps{nh}")
            for nh in range(2)
        ]
        for k in range(KT + 1):
            rows = 128 if k < KT - 1 else (80 if k == KT - 1 else 1)
            rhs_t = xt_t[k] if k < KT else ones_t
            for nh in range(2):
                nc.tensor.matmul(
                    h1ps[nh][:],
                    lhsT=m1_v[0:rows, k, :],
                    rhs=rhs_t[0:rows, 512 * nh : 512 * nh + 512],
                    start=(k == 0),
                    stop=(k == KT),
                )

        # ---- epilogue: leaky(v)=max(v,0.01v) straight from PSUM; ones rows
        h1 = stile([HID + 1, BPC], f32, "h1")
        nc.vector.memset(h1[0 : HID + 1, :], 1.0)
        h2 = stile([HID2 + 1, BPC], f32, "h2")
        nc.vector.memset(h2[0 : HID2 + 1, :], 1.0)
        y_sb = stile([1, BPC], f32, "y")
        pse = ctx.enter_context(tc.tile_pool(name="pse", bufs=1, space="PSUM"))
        h2ps = [
            pse.tile([HID2, 512], f32, tag=f"h2ps{nh}", name=f"h2ps{nh}")
            for nh in range(2)
        ]
        yps = [
            pse.tile([1, 512], f32, tag=f"yps{nh}", name=f"yps{nh}")
            for nh in range(2)
        ]
        h1a = stile([HID, BPC], f32, "h1a")
        h2a = stile([HID2, BPC], f32, "h2a")
        for nh in range(2):
            sl5 = slice(512 * nh, 512 * nh + 512)
            nc.scalar.copy(out=h1a[:, sl5], in_=h1ps[nh][:])
            nc.vector.scalar_tensor_tensor(
                out=h1[0:HID, sl5],
                in0=h1a[:, sl5],
                scalar=0.01,
                in1=h1a[:, sl5],
                op0=op.mult,
                op1=op.max,
            )
            nc.tensor.matmul(
                h2ps[nh][:],
                lhsT=w2ext,
                rhs=h1[0 : HID + 1, sl5],
                start=True,
                stop=True,
            )
            nc.scalar.copy(out=h2a[:, sl5], in_=h2ps[nh][:])
            nc.vector.scalar_tensor_tensor(
                out=h2[0:HID2, sl5],
                in0=h2a[:, sl5],
                scalar=0.01,
                in1=h2a[:, sl5],
                op0=op.mult,
                op1=op.max,
            )
            nc.tensor.matmul(
                yps[nh][:],
                lhsT=w3ext,
                rhs=h2[0 : HID2 + 1, sl5],
                start=True,
                stop=True,
            )
            nc.scalar.copy(out=y_sb[:, sl5], in_=yps[nh][:])
        nc.scalar.dma_start(
            out=out[:].rearrange("(a b) -> a b", a=1), in_=y_sb[:]
        )

    return nc


# ------------------------------------------------------------------- driver
TRACE = False          # set by test harness to capture a profile
LAST_RESULT = None     # BassKernelResults of the last run (when TRACE)


def kernel(x, raw_index, W1, b1, W2, b2, W3, b3):
    global LAST_RESULT
    from concourse.bass_utils import run_bass_kernel_spmd

    x = np.ascontiguousarray(x, np.float32)
    KB, _ = _band_table()
    nc = _build_nc()
    if not nc.is_finalized():
        nc.finalize()
    packed = _pack_small(
        np.asarray(raw_index, np.float32),
        np.asarray(W1, np.float32),
        np.asarray(b1, np.float32),
        np.asarray(W2, np.float32),
        np.asarray(b2, np.float32),
        np.asarray(W3, np.float32),
        np.asarray(b3, np.float32),
    )
    in_maps = []
    for p in range(NCORES):
        in_maps.append(
            {
                "xt": np.ascontiguousarray(x[BPC * p : BPC * (p + 1)].T),
                "kb": KB,
                "small": packed,
            }
        )
    res = run_bass_kernel_spmd(
        nc, in_maps, core_ids=list(range(NCORES)), trace=TRACE
    )
    if TRACE:
        LAST_RESULT = res
    return np.concatenate(
        [np.asarray(res.results[p]["out"]).ravel() for p in range(NCORES)]
    )
